# revision 9
# baseline (speedup 1.0000x reference)
"""GCNBlock (GCNConv + Dropout(eval) + ReLU) Trainium2 kernel, 8 NeuronCores.

Math: out = relu(D^-1/2 (A+I) D^-1/2 (x @ W) + b)
Factorization (aggregate-before-transform):
    out[d] = relu( dinv[d] * ( sum_{s in N(d) u {d}} dinv[s] * x[s] ) @ W + b )

Design:
  * Sources are deduplicated per destination tile and the edge-row stream is
    PRE-GATHERED ON THE HOST into a per-core HBM array laid out
    [128 partitions, chunk, 1024], so the device streams it with plain
    contiguous HWDGE DMA (~16 KB per partition descriptor).  A previous
    dma_gather-based version spent ~120us/core generating SWDGE descriptors
    on the GpSimd engine, which serialized the whole pipeline.
  * Stream rows are fp8 e3m4 with a per-row power-of-two scale 2^k chosen so
    the row max lands in [4, 8): the 4 mantissa bits stay in the normal range
    (measured rel err 1.28e-2 vs the 2e-2 gate).  The un-scale 2^-k is folded
    into the selector entries (powers of two and small multiples are EXACT in
    fp8e3).  PE scatter-accumulates per 128-row chunk:
        psum[d, f] += sel[r, d] * stream[r, f]      (sel = m * 2^-k, binary-ish)
  * The 80 destination tiles are dealt to (core, slot) by sorted unique-source
    count, so the compile-time chunk count per slot is tight and per-core work
    is balanced.
  * Self-loop rows stay fp16 (prescaled dinv[d]*x[d], contiguous DMA) and are
    accumulated with an exact identity selector; y and W use fp16 (full-rate
    on PE, 20x less error than bf16).  Then per dst tile: y *= dinv[dst]
    (ACT), y.T via PE transposes, out = y @ W (PE, W resident), += b, relu.
"""

import sys

import ml_dtypes
import numpy as np

if "/opt/trn_rl_repo" not in sys.path:
    sys.path.insert(0, "/opt/trn_rl_repo")

N_NODES = 10000
DIM = 1024
N_CORES = 8
P = 128
TILES_PER_CORE = 10                      # 10240 padded rows / 8 cores / 128
N_PAD = N_CORES * TILES_PER_CORE * P     # 10240
ROWS_PER_CORE = TILES_PER_CORE * P       # 1280
TOT_TILES = N_PAD // P                   # 80


def _host_preprocess(x, edge_index):
    """Group edges by destination tile, dedup sources per tile, build the
    pre-gathered fp8 stream + selector tables. Returns (layout, *tables)."""
    src = np.asarray(edge_index[0], dtype=np.int64)
    dst = np.asarray(edge_index[1], dtype=np.int64)
    n = N_NODES
    deg = np.bincount(dst, minlength=n).astype(np.float64) + 1.0
    dinv = (1.0 / np.sqrt(deg)).astype(np.float32)

    x_np = np.asarray(x, dtype=np.float32)
    xpre = dinv[:, None] * x_np                      # dinv[s] * x[s]
    rowmax = np.abs(xpre).max(axis=1)
    rowmax = np.where(rowmax > 0, rowmax, 1.0)
    k = np.clip(np.floor(np.log2(8.0 / rowmax)), 0, 6).astype(np.int32)
    selval = (2.0 ** (-k)).astype(np.float32)        # exact in fp8e3

    xq = np.zeros((n + 1, DIM), ml_dtypes.float8_e3m4)   # last row = pad zeros
    xq[:n] = (xpre * (2.0 ** k)[:, None]).astype(ml_dtypes.float8_e3m4)
    xs16 = np.zeros((N_PAD, DIM), np.float16)
    xs16[:n] = xpre.astype(np.float16)               # self rows, fp16
    dinv_pad = np.zeros(N_PAD, np.float32)
    dinv_pad[:n] = dinv

    order = np.argsort(dst, kind="stable")
    s_sorted = src[order]
    d_sorted = dst[order]
    bounds = np.searchsorted(d_sorted, np.arange(0, N_PAD + 1, P))

    # per-tile dedup: unique sources + selector entries (upos, dloc) -> val
    uniqs, entries, u_cnt = [], [], np.zeros(TOT_TILES, np.int64)
    for t in range(TOT_TILES):
        e0, e1 = bounds[t], bounds[t + 1]
        st = s_sorted[e0:e1]
        dt_loc = (d_sorted[e0:e1] - t * P).astype(np.int64)
        uniq, inv = np.unique(st, return_inverse=True)
        uniqs.append(uniq)
        entries.append((inv, dt_loc, selval[st]))
        u_cnt[t] = len(uniq)

    # deal tiles to (core, slot): slot s takes ranks [8s, 8s+8) by count desc,
    # within a slot greedily balance per-core totals
    rank = np.argsort(-u_cnt, kind="stable")
    assign = np.zeros((N_CORES, TILES_PER_CORE), np.int64)
    totals = np.zeros(N_CORES, np.int64)
    for s in range(TILES_PER_CORE):
        tiles_s = rank[s * N_CORES:(s + 1) * N_CORES]
        cores = np.argsort(totals, kind="stable")       # lightest core first
        for j, c in enumerate(cores):
            assign[c, s] = tiles_s[j]                    # biggest to lightest
            totals[c] += u_cnt[tiles_s[j]]

    C_slot = []
    for s in range(TILES_PER_CORE):
        umax = int(u_cnt[assign[:, s]].max())
        C_slot.append(max(1, -(-umax // P)))
    CT = sum(C_slot)
    sel_cols = CT * P

    xg_tbl = np.zeros((N_CORES, P, CT * DIM), ml_dtypes.float8_e3m4)
    sel_tbl = np.zeros((N_CORES, P, sel_cols), ml_dtypes.float8_e3m4)
    dd_tbl = np.zeros((N_CORES, P, TILES_PER_CORE), np.float32)
    xs_tbl = np.zeros((N_CORES, ROWS_PER_CORE, DIM), np.float16)

    for c in range(N_CORES):
        scol = 0
        coff = 0
        for s in range(TILES_PER_CORE):
            t = int(assign[c, s])
            uniq = uniqs[t]
            u = len(uniq)
            C = C_slot[s]
            ids = np.full(C * P, n, np.int64)            # pad -> zero row
            ids[:u] = uniq
            # stream layout: [partition, chunk, feature]
            stream = xq[ids].reshape(C, P, DIM).transpose(1, 0, 2)
            xg_tbl[c, :, coff * DIM:(coff + C) * DIM] = stream.reshape(P, C * DIM)
            coff += C
            # selector block [C*P rows, P dst] -> [P part, C*P cols]
            M = np.zeros((C * P, P), np.float32)
            inv, dloc, val = entries[t]
            np.add.at(M, (inv, dloc), val)
            Mq = M.astype(ml_dtypes.float8_e3m4).reshape(C, P, P)
            sel_tbl[c, :, scol:scol + C * P] = (
                np.transpose(Mq, (1, 0, 2)).reshape(P, C * P))
            scol += C * P
            dd_tbl[c, :, s] = dinv_pad[t * P:(t + 1) * P]
            xs_tbl[c, s * P:(s + 1) * P] = xs16[t * P:(t + 1) * P]

    layout = dict(C=C_slot, CT=CT, sel_cols=sel_cols, assign=assign.tolist())
    return layout, xg_tbl, xs_tbl, sel_tbl, dd_tbl


def _build_bass(layout):
    import concourse.bass as bass  # noqa: F401
    import concourse.mybir as mybir
    import concourse.tile as tile
    from concourse import bacc

    dt = mybir.dt
    C_slot = layout["C"]
    CT = layout["CT"]
    C_max = max(C_slot)
    T = TILES_PER_CORE
    KD = DIM // P  # 8 k-chunks

    nc = bacc.Bacc("TRN2", target_bir_lowering=False, debug=False,
                   num_devices=N_CORES)

    xg_d = nc.dram_tensor("xg", [P, CT * DIM], dt.float8e3, kind="ExternalInput").ap()
    xs_d = nc.dram_tensor("xs", [ROWS_PER_CORE, DIM], dt.float16, kind="ExternalInput").ap()
    w_d = nc.dram_tensor("w", [DIM, DIM], dt.float16, kind="ExternalInput").ap()
    b_d = nc.dram_tensor("b", [1, DIM], dt.float32, kind="ExternalInput").ap()
    sel_d = nc.dram_tensor("sel", [P, layout["sel_cols"]], dt.float8e3, kind="ExternalInput").ap()
    dd_d = nc.dram_tensor("dd", [P, T], dt.float32, kind="ExternalInput").ap()
    out_d = nc.dram_tensor("out", [ROWS_PER_CORE, DIM], dt.float32,
                           kind="ExternalOutput").ap()

    with tile.TileContext(nc) as tc:
        with (
            tc.tile_pool(name="consts", bufs=1) as consts,
            tc.tile_pool(name="g", bufs=3) as gp,
            tc.tile_pool(name="sel", bufs=3) as selp,
            tc.tile_pool(name="xs", bufs=3) as xsp,
            tc.tile_pool(name="y", bufs=2) as ypool,
            tc.tile_pool(name="o", bufs=2) as opool,
            tc.tile_pool(name="psy", bufs=2, space="PSUM") as ps_y,
            tc.tile_pool(name="pstr", bufs=2, space="PSUM") as ps_tr,
            tc.tile_pool(name="pso", bufs=1, space="PSUM") as ps_o,
        ):
            from concourse.masks import make_identity
            eye_sb = consts.tile([P, P], dt.float16)
            make_identity(nc, eye_sb[:])
            w_sb = consts.tile([P, KD, DIM], dt.float16)
            dd_sb = consts.tile([P, T], dt.float32)
            b_sb = consts.tile([1, DIM], dt.float32)
            b_rep = consts.tile([P, DIM], dt.float32)

            coff = [0]
            scol = [0]

            def emit_inputs(s):
                """Issue the input DMAs for slot s; returns the tiles."""
                C = C_slot[s]
                sel_sb = selp.tile([P, C_max * P], dt.float8e3, tag="selblk")
                nc.sync.dma_start(sel_sb[:, 0:C * P],
                                  sel_d[:, scol[0]:scol[0] + C * P])
                scol[0] += C * P
                xs_t = xsp.tile([P, DIM], dt.float16, tag="xs")
                nc.sync.dma_start(xs_t[:], xs_d[s * P:(s + 1) * P, :])
                # pre-gathered fp8 edge-row stream for this slot, DMA'd in
                # 4-chunk pieces so the first scatter matmul only waits on
                # the first 512 KB, not the whole slot stream
                g_sb = gp.tile([P, C_max, DIM], dt.float8e3, tag="g")
                for p0 in range(0, C, 4):
                    p1 = min(p0 + 4, C)
                    nc.sync.dma_start(
                        g_sb[:, p0:p1, :],
                        xg_d[:, (coff[0] + p0) * DIM:(coff[0] + p1) * DIM]
                        .rearrange("p (c f) -> p c f", f=DIM))
                coff[0] += C
                return sel_sb, xs_t, g_sb

            def emit_scatter(s, tiles):
                """PSUM accumulation for slot s; returns (psum_y, y_sb)."""
                C = C_slot[s]
                sel_sb, xs_t, g_sb = tiles
                psum_y = ps_y.tile([P, DIM], dt.float32, tag="py")
                # self-loop: psum_y = I @ xs rows (prescaled dinv[d]*x[d])
                nc.tensor.matmul(psum_y[:, 0:512], eye_sb[:], xs_t[:, 0:512],
                                 start=True, stop=False)
                nc.tensor.matmul(psum_y[:, 512:1024], eye_sb[:],
                                 xs_t[:, 512:1024], start=True, stop=False)
                for ch in range(C):
                    last = (ch == C - 1)
                    sl = sel_sb[:, ch * P:(ch + 1) * P]
                    nc.tensor.matmul(psum_y[:, 0:512], sl,
                                     g_sb[:, ch, 0:512],
                                     start=False, stop=last)
                    nc.tensor.matmul(psum_y[:, 512:1024], sl,
                                     g_sb[:, ch, 512:1024],
                                     start=False, stop=last)
                # y = dinv[dst] * psum  (ACT copy w/ per-partition scale)
                y_sb = ypool.tile([P, DIM], dt.float16, tag="y")
                nc.scalar.mul(y_sb[:], psum_y[:], dd_sb[:, s:s + 1])
                return y_sb

            def emit_transform(s, y_sb):
                """y.T via PE transposes, out = y @ W + b, relu, store."""
                yT = ypool.tile([P, KD, P], dt.float16, tag="yT")
                ps_t = ps_tr.tile([P, KD, P], dt.float16, tag="tr")
                for kc in range(KD):
                    nc.tensor.transpose(ps_t[:, kc, :],
                                        y_sb[:, kc * P:(kc + 1) * P],
                                        eye_sb[:])
                nc.vector.tensor_copy(out=yT[:], in_=ps_t[:])
                ps_out = ps_o.tile([P, DIM], dt.float32, tag="po")
                for kc in range(KD):
                    nc.tensor.matmul(ps_out[:, 0:512], yT[:, kc, :],
                                     w_sb[:, kc, 0:512],
                                     start=(kc == 0), stop=(kc == KD - 1))
                    nc.tensor.matmul(ps_out[:, 512:1024], yT[:, kc, :],
                                     w_sb[:, kc, 512:1024],
                                     start=(kc == 0), stop=(kc == KD - 1))
                o_sb = opool.tile([P, DIM], dt.float32, tag="o")
                nc.vector.tensor_tensor(o_sb[:], ps_out[:], b_rep[:],
                                        mybir.AluOpType.add)
                nc.scalar.activation(o_sb[:], o_sb[:],
                                     mybir.ActivationFunctionType.Relu)
                nc.sync.dma_start(out_d[s * P:(s + 1) * P, :], o_sb[:])

            # slots 0/1 input DMAs go first so the PE can start immediately;
            # the W load and other consts queue behind them (first needed at
            # slot 0's transform, which is ~300 matmuls away)
            tiles0 = emit_inputs(0)
            tiles1 = emit_inputs(1)
            nc.sync.dma_start(w_sb[:], w_d.rearrange("(ko ki) f -> ki ko f", ki=P))
            nc.sync.dma_start(dd_sb[:], dd_d[:])
            nc.sync.dma_start(b_sb[:], b_d[:])
            nc.gpsimd.partition_broadcast(b_rep[:], b_sb[:])

            # software pipeline: scatter(s+1) is emitted before transform(s)
            # so the PE (in-order) never waits on the ACT y-scale latency
            pending = [None, None]              # y_sb for slots s-1, s
            tiles = {0: tiles0, 1: tiles1}
            for s in range(T):
                y_sb = emit_scatter(s, tiles.pop(s))
                if s + 2 < T:
                    tiles[s + 2] = emit_inputs(s + 2)
                if s >= 1:
                    emit_transform(s - 1, pending[1])
                pending = [pending[1], y_sb]
            emit_transform(T - 1, pending[1])

    nc.compile()
    return nc


def _make_in_maps(x, W, b, layout, xg_tbl, xs_tbl, sel_tbl, dd_tbl):
    w_np = np.ascontiguousarray(np.asarray(W, dtype=np.float32).astype(np.float16))
    b_np = np.ascontiguousarray(np.asarray(b, dtype=np.float32)).reshape(1, DIM)
    in_maps = []
    for c in range(N_CORES):
        in_maps.append({
            "xg": np.ascontiguousarray(xg_tbl[c]), "w": w_np, "b": b_np,
            "xs": np.ascontiguousarray(xs_tbl[c]),
            "sel": np.ascontiguousarray(sel_tbl[c]),
            "dd": np.ascontiguousarray(dd_tbl[c]),
        })
    return in_maps


def _assemble(results, layout):
    assign = np.asarray(layout["assign"])
    full = np.zeros((N_PAD, DIM), np.float32)
    for c in range(N_CORES):
        out_c = results[c]["out"]
        for s in range(TILES_PER_CORE):
            t = int(assign[c, s])
            full[t * P:(t + 1) * P] = out_c[s * P:(s + 1) * P]
    return np.ascontiguousarray(full[:N_NODES])


def kernel(x, edge_index, W, b):
    from concourse import bass_utils

    layout, *tbls = _host_preprocess(x, edge_index)
    nc = _build_bass(layout)
    in_maps = _make_in_maps(x, W, b, layout, *tbls)
    res = bass_utils.run_bass_kernel_spmd(nc, in_maps, core_ids=list(range(N_CORES)))
    return _assemble(res.results, layout)


# revision 13
# speedup vs baseline: 1.0694x; 1.0694x over previous
"""GCNBlock (GCNConv + Dropout(eval) + ReLU) Trainium2 kernel, 8 NeuronCores.

Math: out = relu(D^-1/2 (A+I) D^-1/2 (x @ W) + b)
Factorization (aggregate-before-transform):
    out[d] = relu( dinv[d] * ( sum_{s in N(d) u {d}} dinv[s] * x[s] ) @ W + b )

Design:
  * Sources are deduplicated per destination tile and the edge-row stream is
    PRE-GATHERED ON THE HOST into a per-core HBM array laid out
    [128 partitions, chunk, 1024], so the device streams it with plain
    contiguous HWDGE DMA (~16 KB per partition descriptor).  A previous
    dma_gather-based version spent ~120us/core generating SWDGE descriptors
    on the GpSimd engine, which serialized the whole pipeline.
  * Stream rows are fp8 e3m4 with a per-row power-of-two scale 2^k chosen so
    the row max lands in [4, 8): the 4 mantissa bits stay in the normal range
    (measured rel err 1.28e-2 vs the 2e-2 gate).  The un-scale 2^-k is folded
    into the selector entries (powers of two and small multiples are EXACT in
    fp8e3).  PE scatter-accumulates per 128-row chunk:
        psum[d, f] += sel[r, d] * stream[r, f]      (sel = m * 2^-k, binary-ish)
  * The 80 destination tiles are dealt to (core, slot) by sorted unique-source
    count, so the compile-time chunk count per slot is tight and per-core work
    is balanced.
  * Self-loop rows stay fp16 (prescaled dinv[d]*x[d], contiguous DMA) and are
    accumulated with an exact identity selector; y and W use fp16 (full-rate
    on PE, 20x less error than bf16).  Then per dst tile: y *= dinv[dst]
    (ACT), y.T via PE transposes, out = y @ W (PE, W resident), += b, relu.
"""

import sys

import ml_dtypes
import numpy as np

if "/opt/trn_rl_repo" not in sys.path:
    sys.path.insert(0, "/opt/trn_rl_repo")

N_NODES = 10000
DIM = 1024
N_CORES = 8
P = 128
TILES_PER_CORE = 10                      # 10240 padded rows / 8 cores / 128
N_PAD = N_CORES * TILES_PER_CORE * P     # 10240
ROWS_PER_CORE = TILES_PER_CORE * P       # 1280
TOT_TILES = N_PAD // P                   # 80


def _host_preprocess(x, edge_index):
    """Group edges by destination tile, dedup sources per tile, build the
    pre-gathered fp8 stream + selector tables. Returns (layout, *tables)."""
    src = np.asarray(edge_index[0], dtype=np.int64)
    dst = np.asarray(edge_index[1], dtype=np.int64)
    n = N_NODES
    deg = np.bincount(dst, minlength=n).astype(np.float64) + 1.0
    dinv = (1.0 / np.sqrt(deg)).astype(np.float32)

    x_np = np.asarray(x, dtype=np.float32)
    xpre = dinv[:, None] * x_np                      # dinv[s] * x[s]
    rowmax = np.abs(xpre).max(axis=1)
    rowmax = np.where(rowmax > 0, rowmax, 1.0)
    k = np.clip(np.floor(np.log2(8.0 / rowmax)), 0, 6).astype(np.int32)
    selval = (2.0 ** (-k)).astype(np.float32)        # exact in fp8e3

    xq = np.zeros((n + 1, DIM), ml_dtypes.float8_e3m4)   # last row = pad zeros
    xq[:n] = (xpre * (2.0 ** k)[:, None]).astype(ml_dtypes.float8_e3m4)
    xs16 = np.zeros((N_PAD, DIM), np.float16)
    xs16[:n] = (dinv[:, None] * xpre).astype(np.float16)  # dinv^2*x self term
    dinv_pad = np.zeros(N_PAD, np.float32)
    dinv_pad[:n] = dinv

    order = np.argsort(dst, kind="stable")
    s_sorted = src[order]
    d_sorted = dst[order]
    bounds = np.searchsorted(d_sorted, np.arange(0, N_PAD + 1, P))

    # per-tile dedup: unique sources + selector entries (upos, dloc) -> val
    uniqs, entries, u_cnt = [], [], np.zeros(TOT_TILES, np.int64)
    for t in range(TOT_TILES):
        e0, e1 = bounds[t], bounds[t + 1]
        st = s_sorted[e0:e1]
        dt_loc = (d_sorted[e0:e1] - t * P).astype(np.int64)
        uniq, inv = np.unique(st, return_inverse=True)
        uniqs.append(uniq)
        entries.append((inv, dt_loc, selval[st]))
        u_cnt[t] = len(uniq)

    # deal tiles to (core, slot): slot s takes ranks [8s, 8s+8) by count desc,
    # within a slot greedily balance per-core totals
    rank = np.argsort(-u_cnt, kind="stable")
    assign = np.zeros((N_CORES, TILES_PER_CORE), np.int64)
    totals = np.zeros(N_CORES, np.int64)
    for s in range(TILES_PER_CORE):
        tiles_s = rank[s * N_CORES:(s + 1) * N_CORES]
        cores = np.argsort(totals, kind="stable")       # lightest core first
        for j, c in enumerate(cores):
            assign[c, s] = tiles_s[j]                    # biggest to lightest
            totals[c] += u_cnt[tiles_s[j]]

    C_slot = []
    for s in range(TILES_PER_CORE):
        umax = int(u_cnt[assign[:, s]].max())
        C_slot.append(max(1, -(-umax // P)))
    CT = sum(C_slot)
    sel_cols = CT * P

    xg_tbl = np.zeros((N_CORES, P, CT * DIM), ml_dtypes.float8_e3m4)
    sel_tbl = np.zeros((N_CORES, P, sel_cols), ml_dtypes.float8_e3m4)
    dd_tbl = np.zeros((N_CORES, P, TILES_PER_CORE), np.float32)
    xs_tbl = np.zeros((N_CORES, ROWS_PER_CORE, DIM), np.float16)

    for c in range(N_CORES):
        scol = 0
        coff = 0
        for s in range(TILES_PER_CORE):
            t = int(assign[c, s])
            uniq = uniqs[t]
            u = len(uniq)
            C = C_slot[s]
            ids = np.full(C * P, n, np.int64)            # pad -> zero row
            ids[:u] = uniq
            # stream layout: [partition, chunk, feature]
            stream = xq[ids].reshape(C, P, DIM).transpose(1, 0, 2)
            xg_tbl[c, :, coff * DIM:(coff + C) * DIM] = stream.reshape(P, C * DIM)
            coff += C
            # selector block [C*P rows, P dst] -> [P part, C*P cols]
            M = np.zeros((C * P, P), np.float32)
            inv, dloc, val = entries[t]
            np.add.at(M, (inv, dloc), val)
            Mq = M.astype(ml_dtypes.float8_e3m4).reshape(C, P, P)
            sel_tbl[c, :, scol:scol + C * P] = (
                np.transpose(Mq, (1, 0, 2)).reshape(P, C * P))
            scol += C * P
            dd_tbl[c, :, s] = dinv_pad[t * P:(t + 1) * P]
            xs_tbl[c, s * P:(s + 1) * P] = xs16[t * P:(t + 1) * P]

    layout = dict(C=C_slot, CT=CT, sel_cols=sel_cols, assign=assign.tolist())
    return layout, xg_tbl, xs_tbl, sel_tbl, dd_tbl


def _build_bass(layout):
    import concourse.bass as bass  # noqa: F401
    import concourse.mybir as mybir
    import concourse.tile as tile
    from concourse import bacc

    dt = mybir.dt
    C_slot = layout["C"]
    CT = layout["CT"]
    C_max = max(C_slot)
    T = TILES_PER_CORE
    KD = DIM // P  # 8 k-chunks

    nc = bacc.Bacc("TRN2", target_bir_lowering=False, debug=False,
                   num_devices=N_CORES)

    xg_d = nc.dram_tensor("xg", [P, CT * DIM], dt.float8e3, kind="ExternalInput").ap()
    xs_d = nc.dram_tensor("xs", [ROWS_PER_CORE, DIM], dt.float16, kind="ExternalInput").ap()
    w_d = nc.dram_tensor("w", [DIM, DIM], dt.float16, kind="ExternalInput").ap()
    b_d = nc.dram_tensor("b", [1, DIM], dt.float32, kind="ExternalInput").ap()
    sel_d = nc.dram_tensor("sel", [P, layout["sel_cols"]], dt.float8e3, kind="ExternalInput").ap()
    dd_d = nc.dram_tensor("dd", [P, T], dt.float32, kind="ExternalInput").ap()
    out_d = nc.dram_tensor("out", [ROWS_PER_CORE, DIM], dt.float32,
                           kind="ExternalOutput").ap()

    with tile.TileContext(nc) as tc:
        with (
            tc.tile_pool(name="consts", bufs=1) as consts,
            tc.tile_pool(name="g", bufs=3) as gp,
            tc.tile_pool(name="sel", bufs=3) as selp,
            tc.tile_pool(name="xs", bufs=3) as xsp,
            tc.tile_pool(name="y", bufs=2) as ypool,
            tc.tile_pool(name="o", bufs=2) as opool,
            tc.tile_pool(name="psy", bufs=2, space="PSUM") as ps_y,
            tc.tile_pool(name="pstr", bufs=2, space="PSUM") as ps_tr,
            tc.tile_pool(name="pso", bufs=1, space="PSUM") as ps_o,
        ):
            from concourse.masks import make_identity
            eye_sb = consts.tile([P, P], dt.float16)
            make_identity(nc, eye_sb[:])
            w_sb = consts.tile([P, KD, DIM], dt.float16)
            dd_sb = consts.tile([P, T], dt.float32)
            b_sb = consts.tile([1, DIM], dt.float32)
            b_rep = consts.tile([P, DIM], dt.float32)

            coff = [0]
            scol = [0]

            def emit_inputs(s):
                """Issue the input DMAs for slot s; returns the tiles."""
                C = C_slot[s]
                sel_sb = selp.tile([P, C_max * P], dt.float8e3, tag="selblk")
                nc.sync.dma_start(sel_sb[:, 0:C * P],
                                  sel_d[:, scol[0]:scol[0] + C * P])
                scol[0] += C * P
                xs_t = xsp.tile([P, DIM], dt.float16, tag="xs")
                nc.sync.dma_start(xs_t[:], xs_d[s * P:(s + 1) * P, :])
                # pre-gathered fp8 edge-row stream for this slot, DMA'd in
                # 4-chunk pieces so the first scatter matmul only waits on
                # the first 512 KB, not the whole slot stream
                g_sb = gp.tile([P, C_max, DIM], dt.float8e3, tag="g")
                for p0 in range(0, C, 4):
                    p1 = min(p0 + 4, C)
                    nc.sync.dma_start(
                        g_sb[:, p0:p1, :],
                        xg_d[:, (coff[0] + p0) * DIM:(coff[0] + p1) * DIM]
                        .rearrange("p (c f) -> p c f", f=DIM))
                coff[0] += C
                return sel_sb, xs_t, g_sb

            def emit_scatter(s, tiles):
                """PSUM accumulation for slot s; returns y_sb."""
                C = C_slot[s]
                sel_sb, xs_t, g_sb = tiles
                psum_y = ps_y.tile([P, DIM], dt.float32, tag="py")
                for ch in range(C):
                    first, last = (ch == 0), (ch == C - 1)
                    sl = sel_sb[:, ch * P:(ch + 1) * P]
                    nc.tensor.matmul(psum_y[:, 0:512], sl,
                                     g_sb[:, ch, 0:512],
                                     start=first, stop=last)
                    nc.tensor.matmul(psum_y[:, 512:1024], sl,
                                     g_sb[:, ch, 512:1024],
                                     start=first, stop=last)
                # y = dinv[dst] * psum  (ACT copy w/ per-partition scale),
                # then += dinv^2 * x[dst]  (self loop, DVE - off the PE)
                y_sb = ypool.tile([P, DIM], dt.float16, tag="y")
                nc.scalar.mul(y_sb[:], psum_y[:], dd_sb[:, s:s + 1])
                nc.vector.tensor_tensor(y_sb[:], y_sb[:], xs_t[:],
                                        mybir.AluOpType.add)
                return y_sb

            def emit_transform(s, y_sb):
                """y.T via PE transposes, out = y @ W + b, relu, store."""
                yT = ypool.tile([P, KD, P], dt.float16, tag="yT")
                for kc in range(KD):
                    ps_t = ps_tr.tile([P, P], dt.float16, tag="tr")
                    nc.tensor.transpose(ps_t[:], y_sb[:, kc * P:(kc + 1) * P],
                                        eye_sb[:])
                    nc.vector.tensor_copy(out=yT[:, kc, :], in_=ps_t[:])
                ps_out = ps_o.tile([P, DIM], dt.float32, tag="po")
                for kc in range(KD):
                    nc.tensor.matmul(ps_out[:, 0:512], yT[:, kc, :],
                                     w_sb[:, kc, 0:512],
                                     start=(kc == 0), stop=(kc == KD - 1))
                    nc.tensor.matmul(ps_out[:, 512:1024], yT[:, kc, :],
                                     w_sb[:, kc, 512:1024],
                                     start=(kc == 0), stop=(kc == KD - 1))
                o_sb = opool.tile([P, DIM], dt.float32, tag="o")
                nc.vector.tensor_tensor(o_sb[:], ps_out[:], b_rep[:],
                                        mybir.AluOpType.add)
                nc.scalar.activation(o_sb[:], o_sb[:],
                                     mybir.ActivationFunctionType.Relu)
                nc.sync.dma_start(out_d[s * P:(s + 1) * P, :], o_sb[:])

            # tiny consts + slots 0/1 input DMAs go first so the PE and ACT
            # can start immediately; the 2 MB W load queues behind them (first
            # needed at slot 0's transform, ~300 matmuls away)
            nc.sync.dma_start(dd_sb[:], dd_d[:])
            nc.sync.dma_start(b_sb[:], b_d[:])
            nc.gpsimd.partition_broadcast(b_rep[:], b_sb[:])
            tiles0 = emit_inputs(0)
            tiles1 = emit_inputs(1)
            nc.sync.dma_start(w_sb[:], w_d.rearrange("(ko ki) f -> ki ko f", ki=P))

            # software pipeline: scatter(s+1) is emitted before transform(s)
            # so the PE (in-order) never waits on the ACT y-scale latency
            pending = [None, None]              # y_sb for slots s-1, s
            tiles = {0: tiles0, 1: tiles1}
            for s in range(T):
                y_sb = emit_scatter(s, tiles.pop(s))
                if s + 2 < T:
                    tiles[s + 2] = emit_inputs(s + 2)
                if s >= 1:
                    emit_transform(s - 1, pending[1])
                pending = [pending[1], y_sb]
            emit_transform(T - 1, pending[1])

    nc.compile()
    return nc


def _make_in_maps(x, W, b, layout, xg_tbl, xs_tbl, sel_tbl, dd_tbl):
    w_np = np.ascontiguousarray(np.asarray(W, dtype=np.float32).astype(np.float16))
    b_np = np.ascontiguousarray(np.asarray(b, dtype=np.float32)).reshape(1, DIM)
    in_maps = []
    for c in range(N_CORES):
        in_maps.append({
            "xg": np.ascontiguousarray(xg_tbl[c]), "w": w_np, "b": b_np,
            "xs": np.ascontiguousarray(xs_tbl[c]),
            "sel": np.ascontiguousarray(sel_tbl[c]),
            "dd": np.ascontiguousarray(dd_tbl[c]),
        })
    return in_maps


def _assemble(results, layout):
    assign = np.asarray(layout["assign"])
    full = np.zeros((N_PAD, DIM), np.float32)
    for c in range(N_CORES):
        out_c = results[c]["out"]
        for s in range(TILES_PER_CORE):
            t = int(assign[c, s])
            full[t * P:(t + 1) * P] = out_c[s * P:(s + 1) * P]
    return np.ascontiguousarray(full[:N_NODES])


def kernel(x, edge_index, W, b):
    from concourse import bass_utils

    layout, *tbls = _host_preprocess(x, edge_index)
    nc = _build_bass(layout)
    in_maps = _make_in_maps(x, W, b, layout, *tbls)
    res = bass_utils.run_bass_kernel_spmd(nc, in_maps, core_ids=list(range(N_CORES)))
    return _assemble(res.results, layout)
